# revision 1
# baseline (speedup 1.0000x reference)
"""Trainium2 Bass kernel for nn_DiscreteNormalization (WiSARD-style weightless NN).

Reference semantics:
    bits = x[conn]                    # [S, N, B] gather of binary x
    addr = sum_j bits[...,j] << j     # [S, N] 12-bit RAM addresses
    out  = memory[s, n, addr[s,n]]    # [S, N] RAM lookup
    votes= sum_s out                  # [N]
    y    = (votes > S/2).astype(f32)  # [N]

memory is 1 GiB but only S*N = 64K cells are read, so both lookups run as
gathers instead of streaming the table. The neuron axis is sharded across the
8 cores (each core owns all 8 sub-nets for its 1024 neurons -> no cross-core
reduction). Partition p of a core owns neurons n = p*8 + n1, n1 in [0,8).

Per core:
  x-gather   via gpsimd.ap_gather from a [128, 8192] replicated f32 copy of x.
             Indices are each partition's own conn row; the op's 16-partition
             wrapped-index semantics replicate each group's gathered stream
             across its 16 partitions, so a fused (diag-mask * 2^j) multiply +
             reduce both selects each partition's diagonal and packs the
             12-bit address in one pass.
  mem-gather via gpsimd.dma_gather of 512 B rows (128 f32; the row index
             p*256 + n1*32 + addr_hi maxes at exactly 32767, the int16
             limit), one call per sub-net. The wrapped int16 index layout is
             produced by a PE permutation matmul (out[q, (n1,phi)] =
             v[16*phi + q%16, n1], replicated across partition groups).
             A one-hot compare against addr_lo then selects the cell.
"""

import numpy as np

import concourse.bacc as bacc
import concourse.bass as bass
import concourse.mybir as mybir
from concourse.bass_utils import run_bass_kernel_spmd
from concourse.tile import TileContext

S, N, B, IB = 8, 8192, 12, 8192
A = 1 << B                    # 4096 cells per neuron
NCORES = 8
NPC = N // NCORES             # 1024 neurons per core
P = 128
NPP = NPC // P                # 8 neurons per partition
SN = S * NPP                  # 64 (s, n1) pairs per partition
ROW = 128                     # f32 elems per gathered memory row (512 B)
RPN = A // ROW                # 32 rows per neuron table
I32 = mybir.dt.int32
I16 = mybir.dt.int16
F32 = mybir.dt.float32
ALU = mybir.AluOpType
AX = mybir.AxisListType

_cache: dict = {}


def build(loop_iters: int | None = None, xg_chunks: int = 2):
    nc = bacc.Bacc("TRN2", debug=False, enable_asserts=False,
                   num_devices=NCORES, enable_partition_id=False)
    x_d = nc.dram_tensor("x", [IB], I32, kind="ExternalInput")
    conn_d = nc.dram_tensor("conn", [S, NPC, B], I32, kind="ExternalInput")
    mem_d = nc.dram_tensor("mem", [S * NPC * A], F32, kind="ExternalInput")
    y_d = nc.dram_tensor("y", [NPC], F32, kind="ExternalOutput")
    xf_d = nc.dram_tensor("xf_scratch", [1, IB], F32, kind="Internal")

    conn_p = conn_d.ap().rearrange("s (p n1) j -> p s n1 j", p=P)
    y_p = y_d.ap().rearrange("(p n1) -> p n1", p=P)
    # [8, 32768, 128]: per-sub-net windows of 512B rows
    mem_rows = mem_d.ap().rearrange("(s r e) -> s r e", s=S, e=ROW)

    csn = SN // xg_chunks          # (s,n1) pairs per x-gather chunk
    gcols = csn * B * 16           # ap_gather out columns per chunk

    with TileContext(nc) as tc:
        with (tc.tile_pool(name="const", bufs=1) as cpool,
              tc.tile_pool(name="work", bufs=2) as pool,
              tc.tile_pool(name="psum", bufs=2, space="PSUM") as ppool):
            # ---- constants ------------------------------------------------
            # W[p, j*16+r] = (r == p%16) * 2^j   (f32, exact)
            w_r = cpool.tile([P, B, 16], I32)
            nc.gpsimd.iota(w_r[:], pattern=[[0, B], [1, 16]], channel_multiplier=0)
            w_pm = cpool.tile([P, 1], I32)
            nc.gpsimd.iota(w_pm[:], pattern=[[0, 1]], channel_multiplier=1)
            nc.vector.tensor_scalar(out=w_pm[:], in0=w_pm[:], scalar1=15,
                                    scalar2=None, op0=ALU.bitwise_and)
            w_i = cpool.tile([P, B, 16], I32)
            nc.vector.tensor_tensor(out=w_i[:], in0=w_r[:],
                                    in1=w_pm[:].to_broadcast([P, B, 16]),
                                    op=ALU.is_equal)
            w_j2 = cpool.tile([P, B, 16], I32)
            nc.gpsimd.iota(w_j2[:], pattern=[[1, B], [0, 16]], channel_multiplier=0)
            nc.vector.tensor_tensor(out=w_i[:], in0=w_i[:], in1=w_j2[:],
                                    op=ALU.logical_shift_left)  # onehot << j
            W = cpool.tile([P, B * 16], F32)
            nc.vector.tensor_copy(out=W[:], in_=w_i[:].rearrange("p a b -> p (a b)"))

            # L128[p, q] = (p%16 == q%16) f32 — PE fold selector
            l_q = cpool.tile([P, P], I32)
            nc.gpsimd.iota(l_q[:], pattern=[[1, P]], channel_multiplier=0)
            nc.vector.tensor_scalar(out=l_q[:], in0=l_q[:], scalar1=15,
                                    scalar2=None, op0=ALU.bitwise_and)
            l_i = cpool.tile([P, P], I32)
            nc.vector.tensor_tensor(out=l_i[:], in0=l_q[:],
                                    in1=w_pm[:].to_broadcast([P, P]),
                                    op=ALU.is_equal)
            L128 = cpool.tile([P, P], F32)
            nc.vector.tensor_copy(out=L128[:], in_=l_i[:])

            # PM[p, phi] = (p//16 == phi) f32
            pm_i = cpool.tile([P, NPP], I32)
            nc.gpsimd.iota(pm_i[:], pattern=[[1, NPP]], channel_multiplier=0)
            pm_p = cpool.tile([P, 1], I32)
            nc.gpsimd.iota(pm_p[:], pattern=[[0, 1]], channel_multiplier=1)
            nc.vector.tensor_scalar(out=pm_p[:], in0=pm_p[:], scalar1=4,
                                    scalar2=None, op0=ALU.logical_shift_right)
            pm_e = cpool.tile([P, NPP], I32)
            nc.vector.tensor_tensor(out=pm_e[:], in0=pm_i[:],
                                    in1=pm_p[:].to_broadcast([P, NPP]),
                                    op=ALU.is_equal)
            PM = cpool.tile([P, NPP], F32)
            nc.vector.tensor_copy(out=PM[:], in_=pm_e[:])

            # basev[p, n1] = p*256 + n1*32 (int)
            basev = cpool.tile([P, NPP], I32)
            nc.gpsimd.iota(basev[:], pattern=[[RPN, NPP]],
                           channel_multiplier=NPP * RPN)

            # Ciota[p, c] = c (f32)
            ci_i = cpool.tile([P, ROW], I32)
            nc.gpsimd.iota(ci_i[:], pattern=[[1, ROW]], channel_multiplier=0)
            Ciota = cpool.tile([P, ROW], F32)
            nc.vector.tensor_copy(out=Ciota[:], in_=ci_i[:])

            # x -> f32 -> DRAM scratch -> broadcast to all 128 partitions
            x_row = cpool.tile([16, IB // 16], I32)
            nc.sync.dma_start(out=x_row[:],
                              in_=x_d.ap().rearrange("(a b) -> a b", a=16))
            xf_row = cpool.tile([16, IB // 16], F32)
            nc.vector.tensor_copy(out=xf_row[:], in_=x_row[:])
            nc.sync.dma_start(out=xf_d.ap().rearrange("o (a b) -> (o a) b", a=16),
                              in_=xf_row[:])
            XT = cpool.tile([P, IB], F32)
            nc.sync.dma_start(out=XT[:], in_=xf_d.ap().to_broadcast([P, IB]))

            # conn -> int16 indices
            CT = cpool.tile([P, SN, B], I32)
            nc.sync.dma_start(out=CT[:], in_=conn_p)
            CT16 = cpool.tile([P, SN * B], I16)
            nc.vector.tensor_copy(out=CT16[:], in_=CT[:].rearrange("p a b -> p (a b)"))

            vals = cpool.tile([P, SN], F32)        # selected cells
            G = cpool.tile([P, SN, ROW], F32)      # gathered 512B rows

            def body(_=None):
                cs = csn // NPP            # sub-nets per x-gather chunk

                def emit_xgather(ch):
                    g = pool.tile([P, gcols], F32, tag="g")
                    nc.gpsimd.ap_gather(
                        out_ap=g[:], in_ap=XT[:],
                        idxs_ap=CT16[:, ch * csn * B:(ch + 1) * csn * B],
                        channels=P, num_elems=IB, d=1, num_idxs=csn * B * 16,
                    )
                    nc.vector.tensor_tensor(
                        out=g[:].rearrange("p (sn w) -> p sn w", w=B * 16),
                        in0=g[:].rearrange("p (sn w) -> p sn w", w=B * 16),
                        in1=W[:][:, None, :].to_broadcast([P, csn, B * 16]),
                        op=ALU.mult)
                    addr_f = pool.tile([P, csn], F32, tag="addr_f")
                    with nc.allow_low_precision(reason="sums < 4096, exact"):
                        nc.vector.tensor_reduce(
                            out=addr_f[:],
                            in_=g[:].rearrange("p (sn w) -> p sn w", w=B * 16),
                            axis=AX.X, op=ALU.add)
                    ai = pool.tile([P, csn], I32, tag="ai", bufs=2)
                    nc.vector.tensor_copy(out=ai[:], in_=addr_f[:])
                    return ai

                def emit_chains(ch, ai):
                    # batched fold + row gather + cell select for one chunk
                    # (cs sub-nets at once); issued one chunk late so Pool
                    # never stalls between ap_gathers
                    ahi = pool.tile([P, cs, NPP], I32, tag="ahi")
                    nc.vector.tensor_scalar(
                        out=ahi[:], in0=ai[:].rearrange("p (a b) -> p a b", b=NPP),
                        scalar1=7, scalar2=None, op0=ALU.logical_shift_right)
                    nc.vector.tensor_tensor(
                        out=ahi[:], in0=ahi[:],
                        in1=basev[:][:, None, :].to_broadcast([P, cs, NPP]),
                        op=ALU.bitwise_or)
                    vf = pool.tile([P, cs, NPP], F32, tag="vf")
                    nc.vector.tensor_copy(out=vf[:], in_=ahi[:])
                    rhs = pool.tile([P, cs, NPP, NPP], F32, tag="rhs")
                    nc.vector.tensor_tensor(
                        out=rhs[:],
                        in0=vf[:][:, :, :, None].to_broadcast([P, cs, NPP, NPP]),
                        in1=PM[:][:, None, None, :].to_broadcast([P, cs, NPP, NPP]),
                        op=ALU.mult)
                    folded = ppool.tile([P, cs * SN], F32, tag="folded",
                                        space="PSUM")
                    nc.tensor.matmul(out=folded[:], lhsT=L128[:],
                                     rhs=rhs[:].rearrange("p a b c -> p (a b c)"),
                                     start=True, stop=True)
                    idx16 = pool.tile([P, cs * SN], I16, tag="idx16")
                    nc.vector.tensor_copy(out=idx16[:], in_=folded[:])
                    for si in range(cs):
                        s = ch * cs + si
                        nc.gpsimd.dma_gather(
                            out_ap=G[:, s * NPP:(s + 1) * NPP],
                            in_ap=mem_rows[s],
                            idxs_ap=idx16[:, si * SN:(si + 1) * SN],
                            num_idxs=P * NPP,
                            num_idxs_reg=P * NPP,
                            elem_size=ROW,
                        )
                    alo = pool.tile([P, csn], I32, tag="alo")
                    nc.vector.tensor_scalar(out=alo[:], in0=ai[:],
                                            scalar1=ROW - 1, scalar2=None,
                                            op0=ALU.bitwise_and)
                    alo_f = pool.tile([P, csn], F32, tag="alo_f")
                    nc.vector.tensor_copy(out=alo_f[:], in_=alo[:])
                    m2 = pool.tile([P, csn, ROW], F32, tag="m2")
                    nc.vector.tensor_tensor(
                        out=m2[:],
                        in0=alo_f[:][:, :, None].to_broadcast([P, csn, ROW]),
                        in1=Ciota[:][:, None, :].to_broadcast([P, csn, ROW]),
                        op=ALU.is_equal)
                    nc.vector.tensor_tensor(
                        out=m2[:], in0=m2[:],
                        in1=G[:, ch * csn:(ch + 1) * csn], op=ALU.mult)
                    nc.vector.tensor_reduce(
                        out=vals[:, ch * csn:(ch + 1) * csn], in_=m2[:],
                        axis=AX.X, op=ALU.add)

                pending = None       # (ch, ai) whose chains are not yet issued
                for ch in range(xg_chunks):
                    ai = emit_xgather(ch)
                    if pending is not None:
                        emit_chains(*pending)
                    pending = (ch, ai)
                emit_chains(*pending)
                votes = pool.tile([P, NPP], F32, tag="votes")
                nc.vector.tensor_reduce(
                    out=votes[:],
                    in_=vals[:].rearrange("p (s n1) -> p n1 s", s=S),
                    axis=AX.X, op=ALU.add)
                res = pool.tile([P, NPP], F32, tag="res")
                nc.vector.tensor_scalar(out=res[:], in0=votes[:],
                                        scalar1=float(S) / 2.0, scalar2=None,
                                        op0=ALU.is_gt)
                nc.sync.dma_start(out=y_p, in_=res[:])

            if loop_iters is None:
                body()
            else:
                with tc.For_i(0, loop_iters, 1) as _i:
                    body(_i)

    nc.compile()
    return nc


def _get(loop_iters=None):
    key = loop_iters
    if key not in _cache:
        _cache[key] = build(loop_iters)
    return _cache[key]


def make_in_maps(x, conn, memory):
    """Slice full inputs into per-core input maps (host-side sharding only)."""
    ins = []
    for c in range(NCORES):
        lo, hi = c * NPC, (c + 1) * NPC
        ins.append({
            "x": np.ascontiguousarray(x).astype(np.int32, copy=False),
            "conn": np.ascontiguousarray(conn[:, lo:hi, :]).astype(
                np.int32, copy=False),
            "mem": np.ascontiguousarray(memory[:, lo:hi, :]).reshape(-1).astype(
                np.float32, copy=False),
        })
    return ins


def kernel(x, conn, memory, *, loop_iters=None):
    nc = _get(loop_iters)
    ins = make_in_maps(x, conn, memory)
    res = run_bass_kernel_spmd(nc, ins, core_ids=list(range(NCORES)))
    return np.concatenate([res.results[c]["y"] for c in range(NCORES)]).astype(
        np.float32)



# revision 3
# speedup vs baseline: 1.9394x; 1.9394x over previous
"""Trainium2 Bass kernel for nn_DiscreteNormalization (WiSARD-style weightless NN).

Reference semantics:
    bits = x[conn]                    # [S, N, B] gather of binary x
    addr = sum_j bits[...,j] << j     # [S, N] 12-bit RAM addresses
    out  = memory[s, n, addr[s,n]]    # [S, N] RAM lookup
    votes= sum_s out                  # [N]
    y    = (votes > S/2).astype(f32)  # [N]

memory is 1 GiB but only S*N = 64K cells are read, so both lookups run as
gathers instead of streaming the table. The neuron axis is sharded across the
8 cores (each core owns all 8 sub-nets for its 1024 neurons -> no cross-core
reduction). Partition p of a core owns neurons n = p*8 + n1, n1 in [0,8).

Per core, per iteration:
  x-gather   via gpsimd.ap_gather (2 chunks of 6144 wrapped idx per 16-
             partition group) from a [128, 8192] replicated f32 copy of x.
             A fused (diag-mask * 2^j) multiply + reduce selects each
             partition's own lanes and packs the 12-bit address.
  mem-gather via gpsimd.dma_gather of 256 B bf16 rows (128 cells; the table
             is converted to bf16 on the host - cells are exactly 0.0/1.0),
             one call per sub-net, round-robined over 4 SWDGE queues so the
             transfers spread across DMA rings. The wrapped int16 row-index
             layout is produced by a PE permutation matmul. A one-hot
             compare against addr&127 (bf16, 2x DVE mode) selects the cell.
  G and vals live in double-buffered pool tiles so consecutive loop
  iterations overlap (no cross-iteration WAR serialization).

Measured per-iteration time (8-core SPMD, steady-state hardware loop):
~397 us vs ~456 us for the previous single-queue f32 version.
"""

import numpy as np
import ml_dtypes

import concourse.bacc as bacc
import concourse.bass as bass
import concourse.mybir as mybir
from concourse.bass_utils import run_bass_kernel_spmd
from concourse.tile import TileContext

S, N, B, IB = 8, 8192, 12, 8192
A = 1 << B                    # 4096 cells per neuron
NCORES = 8
NPC = N // NCORES             # 1024 neurons per core
P = 128
NPP = NPC // P                # 8 neurons per partition
SN = S * NPP                  # 64 (s, n1) pairs per partition
ROW = 128                     # bf16 cells per gathered memory row (256 B)
RPN = A // ROW                # 32 rows per neuron table
CHUNKS = [6, 2]               # sub-nets per ap_gather chunk (asymmetric:
                              # small last chunk -> short exposed fold stall)
NQUEUES = 4                   # SWDGE queues for dma_gather round-robin
I32 = mybir.dt.int32
I16 = mybir.dt.int16
F32 = mybir.dt.float32
BF16 = mybir.dt.bfloat16
ALU = mybir.AluOpType
AX = mybir.AxisListType

_cache: dict = {}


def build(loop_iters: int | None = None):
    nc = bacc.Bacc("TRN2", debug=False, enable_asserts=False,
                   num_devices=NCORES, enable_partition_id=False,
                   num_swdge_queues=NQUEUES)
    x_d = nc.dram_tensor("x", [IB], I32, kind="ExternalInput")
    conn_d = nc.dram_tensor("conn", [S, NPC, B], I32, kind="ExternalInput")
    mem_d = nc.dram_tensor("mem", [S * NPC * A], BF16, kind="ExternalInput")
    y_d = nc.dram_tensor("y", [NPC], F32, kind="ExternalOutput")
    xf_d = nc.dram_tensor("xf_scratch", [1, IB], F32, kind="Internal")

    conn_p = conn_d.ap().rearrange("s (p n1) j -> p s n1 j", p=P)
    y_p = y_d.ap().rearrange("(p n1) -> p n1", p=P)
    # [8, 32768, 128]: per-sub-net windows of 256 B bf16 rows
    mem_rows = mem_d.ap().rearrange("(s r e) -> s r e", s=S, e=ROW)

    assert sum(CHUNKS) == S

    with TileContext(nc) as tc:
        with (tc.tile_pool(name="const", bufs=1) as cpool,
              tc.tile_pool(name="work", bufs=2) as pool,
              tc.tile_pool(name="psum", bufs=2, space="PSUM") as ppool):
            # ---- constants ------------------------------------------------
            # W[p, j*16+r] = (r == p%16) * 2^j   (f32, exact)
            w_r = cpool.tile([P, B, 16], I32)
            nc.gpsimd.iota(w_r[:], pattern=[[0, B], [1, 16]], channel_multiplier=0)
            w_pm = cpool.tile([P, 1], I32)
            nc.gpsimd.iota(w_pm[:], pattern=[[0, 1]], channel_multiplier=1)
            nc.vector.tensor_scalar(out=w_pm[:], in0=w_pm[:], scalar1=15,
                                    scalar2=None, op0=ALU.bitwise_and)
            w_i = cpool.tile([P, B, 16], I32)
            nc.vector.tensor_tensor(out=w_i[:], in0=w_r[:],
                                    in1=w_pm[:].to_broadcast([P, B, 16]),
                                    op=ALU.is_equal)
            w_j2 = cpool.tile([P, B, 16], I32)
            nc.gpsimd.iota(w_j2[:], pattern=[[1, B], [0, 16]], channel_multiplier=0)
            nc.vector.tensor_tensor(out=w_i[:], in0=w_i[:], in1=w_j2[:],
                                    op=ALU.logical_shift_left)  # onehot << j
            W = cpool.tile([P, B * 16], F32)
            nc.vector.tensor_copy(out=W[:], in_=w_i[:].rearrange("p a b -> p (a b)"))

            # L128[p, q] = (p%16 == q%16) f32 -- PE fold selector
            l_q = cpool.tile([P, P], I32)
            nc.gpsimd.iota(l_q[:], pattern=[[1, P]], channel_multiplier=0)
            nc.vector.tensor_scalar(out=l_q[:], in0=l_q[:], scalar1=15,
                                    scalar2=None, op0=ALU.bitwise_and)
            l_i = cpool.tile([P, P], I32)
            nc.vector.tensor_tensor(out=l_i[:], in0=l_q[:],
                                    in1=w_pm[:].to_broadcast([P, P]),
                                    op=ALU.is_equal)
            L128 = cpool.tile([P, P], F32)
            nc.vector.tensor_copy(out=L128[:], in_=l_i[:])

            # PM[p, phi] = (p//16 == phi) f32
            pm_i = cpool.tile([P, NPP], I32)
            nc.gpsimd.iota(pm_i[:], pattern=[[1, NPP]], channel_multiplier=0)
            pm_p = cpool.tile([P, 1], I32)
            nc.gpsimd.iota(pm_p[:], pattern=[[0, 1]], channel_multiplier=1)
            nc.vector.tensor_scalar(out=pm_p[:], in0=pm_p[:], scalar1=4,
                                    scalar2=None, op0=ALU.logical_shift_right)
            pm_e = cpool.tile([P, NPP], I32)
            nc.vector.tensor_tensor(out=pm_e[:], in0=pm_i[:],
                                    in1=pm_p[:].to_broadcast([P, NPP]),
                                    op=ALU.is_equal)
            PM = cpool.tile([P, NPP], F32)
            nc.vector.tensor_copy(out=PM[:], in_=pm_e[:])

            # basev[p, n1] = (p*NPP + n1) * RPN  (row base per neuron)
            basev = cpool.tile([P, NPP], I32)
            nc.gpsimd.iota(basev[:], pattern=[[RPN, NPP]],
                           channel_multiplier=NPP * RPN)

            # Ciota[p, c] = c (bf16)
            ci_i = cpool.tile([P, ROW], I32)
            nc.gpsimd.iota(ci_i[:], pattern=[[1, ROW]], channel_multiplier=0)
            Ciota = cpool.tile([P, ROW], BF16)
            nc.vector.tensor_copy(out=Ciota[:], in_=ci_i[:])

            # x -> f32 -> DRAM scratch -> broadcast to all 128 partitions
            x_row = cpool.tile([16, IB // 16], I32)
            nc.sync.dma_start(out=x_row[:],
                              in_=x_d.ap().rearrange("(a b) -> a b", a=16))
            xf_row = cpool.tile([16, IB // 16], F32)
            nc.vector.tensor_copy(out=xf_row[:], in_=x_row[:])
            nc.sync.dma_start(out=xf_d.ap().rearrange("o (a b) -> (o a) b", a=16),
                              in_=xf_row[:])
            XT = cpool.tile([P, IB], F32)
            nc.sync.dma_start(out=XT[:], in_=xf_d.ap().to_broadcast([P, IB]))

            # conn -> int16 indices
            CT = cpool.tile([P, SN, B], I32)
            nc.sync.dma_start(out=CT[:], in_=conn_p)
            CT16 = cpool.tile([P, SN * B], I16)
            nc.vector.tensor_copy(out=CT16[:], in_=CT[:].rearrange("p a b -> p (a b)"))

            def body(_=None):
                # double-buffered across iterations: no cross-iter WAR stalls
                vals = pool.tile([P, SN], BF16, tag="vals", bufs=2)
                G = pool.tile([P, SN, ROW], BF16, tag="G", bufs=2)

                def emit_xgather(ch, s0, cs):
                    csn = cs * NPP
                    g = pool.tile([P, csn * B * 16], F32, tag=f"g{ch}",
                                  bufs=1)
                    nc.gpsimd.ap_gather(
                        out_ap=g[:], in_ap=XT[:],
                        idxs_ap=CT16[:, s0 * NPP * B:(s0 + cs) * NPP * B],
                        channels=P, num_elems=IB, d=1, num_idxs=csn * B * 16,
                    )
                    nc.vector.tensor_tensor(
                        out=g[:].rearrange("p (sn w) -> p sn w", w=B * 16),
                        in0=g[:].rearrange("p (sn w) -> p sn w", w=B * 16),
                        in1=W[:][:, None, :].to_broadcast([P, csn, B * 16]),
                        op=ALU.mult)
                    addr_f = pool.tile([P, csn], F32, tag=f"addr_f{ch}",
                                       bufs=1)
                    with nc.allow_low_precision(reason="sums < 4096, exact"):
                        nc.vector.tensor_reduce(
                            out=addr_f[:],
                            in_=g[:].rearrange("p (sn w) -> p sn w", w=B * 16),
                            axis=AX.X, op=ALU.add)
                    ai = pool.tile([P, csn], I32, tag=f"ai{ch}", bufs=1)
                    nc.vector.tensor_copy(out=ai[:], in_=addr_f[:])
                    return ai

                def emit_chains(ch, s0, cs, ai):
                    # batched fold + row gather + cell select for one chunk,
                    # issued one chunk late so Pool never stalls between
                    # ap_gathers
                    csn = cs * NPP
                    ahi = pool.tile([P, cs, NPP], I32, tag=f"ahi{ch}", bufs=1)
                    nc.vector.tensor_scalar(
                        out=ahi[:], in0=ai[:].rearrange("p (a b) -> p a b", b=NPP),
                        scalar1=7, scalar2=None, op0=ALU.logical_shift_right)
                    nc.vector.tensor_tensor(
                        out=ahi[:], in0=ahi[:],
                        in1=basev[:][:, None, :].to_broadcast([P, cs, NPP]),
                        op=ALU.bitwise_or)
                    vf = pool.tile([P, cs, NPP], F32, tag=f"vf{ch}", bufs=1)
                    nc.vector.tensor_copy(out=vf[:], in_=ahi[:])
                    rhs = pool.tile([P, cs, NPP, NPP], F32, tag=f"rhs{ch}",
                                    bufs=1)
                    nc.vector.tensor_tensor(
                        out=rhs[:],
                        in0=vf[:][:, :, :, None].to_broadcast([P, cs, NPP, NPP]),
                        in1=PM[:][:, None, None, :].to_broadcast([P, cs, NPP, NPP]),
                        op=ALU.mult)
                    folded = ppool.tile([P, cs * SN], F32, tag=f"folded{ch}",
                                        space="PSUM")
                    nc.tensor.matmul(out=folded[:], lhsT=L128[:],
                                     rhs=rhs[:].rearrange("p a b c -> p (a b c)"),
                                     start=True, stop=True)
                    idx16 = pool.tile([P, cs * SN], I16, tag=f"idx16{ch}",
                                      bufs=1)
                    nc.vector.tensor_copy(out=idx16[:], in_=folded[:])
                    for si in range(cs):
                        s = s0 + si
                        nc.gpsimd.dma_gather(
                            out_ap=G[:, s * NPP:(s + 1) * NPP],
                            in_ap=mem_rows[s],
                            idxs_ap=idx16[:, si * SN:(si + 1) * SN],
                            num_idxs=P * NPP,
                            num_idxs_reg=P * NPP,
                            elem_size=ROW,
                            queue_num=s % NQUEUES,
                        )
                    alo = pool.tile([P, csn], I32, tag=f"alo{ch}", bufs=1)
                    nc.vector.tensor_scalar(out=alo[:], in0=ai[:],
                                            scalar1=ROW - 1, scalar2=None,
                                            op0=ALU.bitwise_and)
                    alo_f = pool.tile([P, csn], BF16, tag=f"alo_f{ch}", bufs=1)
                    nc.vector.tensor_copy(out=alo_f[:], in_=alo[:])
                    m2 = pool.tile([P, csn, ROW], BF16, tag=f"m2{ch}", bufs=1)
                    nc.vector.tensor_tensor(
                        out=m2[:],
                        in0=alo_f[:][:, :, None].to_broadcast([P, csn, ROW]),
                        in1=Ciota[:][:, None, :].to_broadcast([P, csn, ROW]),
                        op=ALU.is_equal)
                    nc.vector.tensor_tensor(
                        out=m2[:], in0=m2[:],
                        in1=G[:, s0 * NPP:(s0 + cs) * NPP], op=ALU.mult)
                    with nc.allow_low_precision(reason="one-hot, sums exact"):
                        nc.vector.tensor_reduce(
                            out=vals[:, s0 * NPP:(s0 + cs) * NPP], in_=m2[:],
                            axis=AX.X, op=ALU.add)

                pending = None       # chunk whose chains are not yet issued
                s0 = 0
                for ch, cs in enumerate(CHUNKS):
                    ai = emit_xgather(ch, s0, cs)
                    if pending is not None:
                        emit_chains(*pending)
                    pending = (ch, s0, cs, ai)
                    s0 += cs
                emit_chains(*pending)
                votes = pool.tile([P, NPP], F32, tag="votes")
                with nc.allow_low_precision(reason="sums <= 8, exact"):
                    nc.vector.tensor_reduce(
                        out=votes[:],
                        in_=vals[:].rearrange("p (s n1) -> p n1 s", s=S),
                        axis=AX.X, op=ALU.add)
                res = pool.tile([P, NPP], F32, tag="res")
                nc.vector.tensor_scalar(out=res[:], in0=votes[:],
                                        scalar1=float(S) / 2.0, scalar2=None,
                                        op0=ALU.is_gt)
                nc.sync.dma_start(out=y_p, in_=res[:])

            if loop_iters is None:
                body()
            else:
                with tc.For_i(0, loop_iters, 1) as _i:
                    body(_i)

    nc.compile()
    return nc


def _get(loop_iters=None):
    key = loop_iters
    if key not in _cache:
        _cache[key] = build(loop_iters)
    return _cache[key]


def make_in_maps(x, conn, memory):
    """Slice full inputs into per-core input maps (host-side staging only).

    The memory table holds exact 0.0/1.0 values, so the bf16 conversion is
    lossless; it halves the HBM gather traffic and enables 2x-rate DVE ops
    for the in-row cell select.
    """
    mem_b = memory.astype(ml_dtypes.bfloat16)
    ins = []
    for c in range(NCORES):
        lo, hi = c * NPC, (c + 1) * NPC
        ins.append({
            "x": np.ascontiguousarray(x).astype(np.int32, copy=False),
            "conn": np.ascontiguousarray(conn[:, lo:hi, :]).astype(
                np.int32, copy=False),
            "mem": np.ascontiguousarray(mem_b[:, lo:hi, :]).reshape(-1),
        })
    return ins


def kernel(x, conn, memory, *, loop_iters=None):
    nc = _get(loop_iters)
    ins = make_in_maps(x, conn, memory)
    res = run_bass_kernel_spmd(nc, ins, core_ids=list(range(NCORES)))
    return np.concatenate([res.results[c]["y"] for c in range(NCORES)]).astype(
        np.float32)
